# revision 6
# baseline (speedup 1.0000x reference)
"""CFM contrastive loss on 8 TRN2 NeuronCores — dual-lane exp version.

loss = -mean(diag(log_softmax(logits))),  logits[i,j] = 2*z1_i.z2_j - |z1_i|^2 - |z2_j|^2

The |z1_i|^2 term cancels between the logsumexp and the diagonal, so with
t[i,j] = 2*z1_i.z2_j - |z2_j|^2 the loss is mean_i(log(sum_j exp(t_ij)) - t_ii).

Sharding: z1 rows split across 8 cores (1024 rows each); every core reads all
of z2.  Per core the 1024x8192 block of t is produced in PSUM by fp16 matmuls
(PSUM holds A*t (+B on DVE granules), A = 128/ln2, pre-scaled on the host into
z1t2 and the bias strips), in 64 granules of [128 rows x 1024 cols].

The exp+rowsum of each granule goes down one of two lanes (the baseline was
ScalarE-bound at ~90% busy, so ~40% of the work moves to VectorE):
  - ACT lane: ScalarE activation Exp (scale=1/A) straight out of PSUM with
    accum_out producing the granule's row-sums.  1 elem/cycle @ 1.2 GHz.
  - DVE lane: Schraudolph integer exp.  PSUM already holds y = A*t + B, which
    is the uint16 bit pattern of bf16(e^t) (B includes the -0.0397-nat bias
    correction).  pass1: tensor_scalar max(y,0) -> uint16 (1x from PSUM); the
    clamp keeps every out-of-range logit at +0.0 so no NaN/negative bitcast
    garbage can appear.  pass2 (per i-tile, batched): bypass over the uint16
    tile bitcast to bf16 at 4x mode with accum_out giving the row-sums.
Rel-err budget is ~2e-2 of a loss of ~104.5 (i.e. +-2 abs), the Schraudolph
worst-case log error is ~0.04, so the approximation is safe by a wide margin.

Host finishes with log + mean in float64 plus the cheap O(N*D) diagonal.
"""

import math

import numpy as np
import ml_dtypes

N, D = 8192, 128
NCORES = 8
SHARD = N // NCORES      # 1024 z1 rows per core
ITILES = SHARD // 128    # 8 i-tiles per core
GRAN = 1024              # granule width (2 PSUM banks)
NGRAN = N // GRAN        # 8 granules of j per i-tile
F16 = ml_dtypes.bfloat16  # overwritten below; fp16 used for operands
F16 = np.float16

A_SCALE = 128.0 / math.log(2.0)        # bits per nat
B_MAGIC = 16256.0 - 0.0397 * A_SCALE   # bf16 exponent bias - Schraudolph shift

N_DVE = 25  # granules (of 64) handled by the DVE lane

_NC_CACHE = None


def _dve_set():
    """Evenly spread N_DVE granule indices over the 64 (it, g) slots."""
    return {int((k + 0.5) * 64.0 / N_DVE) for k in range(N_DVE)}


def _build_nc():
    import concourse.mybir as mybir
    import concourse.tile as tile
    from concourse import bacc

    nc = bacc.Bacc(None, target_bir_lowering=False)

    z1t2 = nc.dram_tensor("z1t2", [128, SHARD], mybir.dt.float16, kind="ExternalInput")
    z2t = nc.dram_tensor("z2t", [128, N], mybir.dt.float16, kind="ExternalInput")
    # strip[r, c*512+u] = bias for psum bank r of chunk c (j = c*2048 + r*512 + u)
    # stripA: A*(-sq2_j)            (ScalarE granules)
    # stripD: A*(-sq2_j) + B_MAGIC  (DVE granules)
    stripA = nc.dram_tensor("stripA", [4, N // 4], mybir.dt.float16, kind="ExternalInput")
    stripD = nc.dram_tensor("stripD", [4, N // 4], mybir.dt.float16, kind="ExternalInput")
    rs_a = nc.dram_tensor("rs_a", [128, ITILES * NGRAN], mybir.dt.float32, kind="ExternalOutput")
    rs_d = nc.dram_tensor("rs_d", [128, ITILES], mybir.dt.float32, kind="ExternalOutput")

    EXP = mybir.ActivationFunctionType.Exp
    dve = _dve_set()

    with tile.TileContext(nc) as tc:
        with (
            tc.tile_pool(name="const", bufs=1) as cpool,
            tc.tile_pool(name="acts", bufs=2) as apool,
            tc.tile_pool(name="bits", bufs=2) as bpool,
            tc.tile_pool(name="psum", bufs=4, space="PSUM") as ppool,
        ):
            z1t2_sb = cpool.tile([128, SHARD], mybir.dt.float16)
            z2t_sb = cpool.tile([128, N], mybir.dt.float16)
            stripA_sb = cpool.tile([128, N // 4], mybir.dt.float16)
            stripD_sb = cpool.tile([128, N // 4], mybir.dt.float16)
            ones_sb = cpool.tile([128, 128], mybir.dt.float16)
            rs_a_sb = cpool.tile([128, ITILES * NGRAN], mybir.dt.float32)
            rs_d_sb = cpool.tile([128, ITILES], mybir.dt.float32)
            warm_sb = cpool.tile([1, 1], mybir.dt.float32)

            # Load the exp table set at t=0, concurrent with the input DMAs.
            nc.scalar.activation(warm_sb[:], warm_sb[:], EXP)

            nc.gpsimd.memset(ones_sb[:], 1.0)
            nc.sync.dma_start(stripA_sb[0:97:32, :], stripA[:, :])
            nc.sync.dma_start(stripD_sb[0:97:32, :], stripD[:, :])
            nc.sync.dma_start(z1t2_sb[:], z1t2[:])
            for q in range(NGRAN):
                nc.sync.dma_start(
                    z2t_sb[:, q * GRAN : (q + 1) * GRAN],
                    z2t[:, q * GRAN : (q + 1) * GRAN],
                )

            for it in range(ITILES):
                lhsT = z1t2_sb[:, it * 128 : (it + 1) * 128]
                slot = 0
                batch = bpool.tile([128, 4 * GRAN], mybir.dt.uint16)
                dummy = bpool.tile([128, 1], mybir.dt.bfloat16)
                for c in range(4):  # chunks of 2048 = 2 granules = 4 banks
                    gpair = (2 * c, 2 * c + 1)
                    ps = [
                        ppool.tile([128, GRAN], mybir.dt.float32, name="ps")
                        for _ in gpair
                    ]
                    # 4 concurrent K=1 matmuls (one per PE row-group) broadcast
                    # the per-j bias strip into the 4 PSUM banks of this chunk.
                    for r in range(4):
                        p0 = 32 * r
                        strip_sb = stripD_sb if (it * 8 + gpair[r // 2]) in dve else stripA_sb
                        nc.tensor.matmul(
                            ps[r // 2][:, (r % 2) * 512 : (r % 2 + 1) * 512],
                            ones_sb[p0 : p0 + 1, :],
                            strip_sb[p0 : p0 + 1, c * 512 : (c + 1) * 512],
                            start=True,
                            stop=False,
                            tile_position=(p0, 0),
                        )
                    for h, g in enumerate(gpair):
                        for b in range(2):
                            j0 = g * GRAN + b * 512
                            nc.tensor.matmul(
                                ps[h][:, b * 512 : (b + 1) * 512],
                                lhsT,
                                z2t_sb[:, j0 : j0 + 512],
                                start=False,
                                stop=True,
                            )
                        if (it * 8 + g) in dve:
                            # Schraudolph pass1: psum already holds A*t+B, the
                            # uint16 pattern of bf16(e^t); clamp negatives to 0.
                            nc.vector.tensor_scalar(
                                batch[:, slot * GRAN : (slot + 1) * GRAN],
                                ps[h][:],
                                0.0,
                                None,
                                op0=mybir.AluOpType.max,
                            )
                            slot += 1
                        else:
                            e_tile = apool.tile([128, GRAN], mybir.dt.bfloat16)
                            col = it * NGRAN + g
                            nc.scalar.activation(
                                e_tile[:],
                                ps[h][:],
                                EXP,
                                bias=0.0,
                                scale=1.0 / A_SCALE,
                                accum_out=rs_a_sb[:, col : col + 1],
                            )
                if slot:
                    # Schraudolph pass2: sum the bf16-bitcast exp values (4x mode).
                    # op1 is the reduce op of the fused TensorScalarPtrReduce.
                    nc.vector.tensor_scalar(
                        dummy.broadcast_to((128, slot * GRAN)),
                        batch[:, : slot * GRAN].bitcast(mybir.dt.bfloat16),
                        0.0,
                        0.0,
                        op0=mybir.AluOpType.bypass,
                        op1=mybir.AluOpType.add,
                        accum_out=rs_d_sb[:, it : it + 1],
                    )
                else:
                    nc.vector.memset(rs_d_sb[:, it : it + 1], 0.0)

            nc.sync.dma_start(rs_a[:], rs_a_sb[:])
            nc.sync.dma_start(rs_d[:], rs_d_sb[:])

    nc.compile()
    return nc


def _get_nc():
    global _NC_CACHE
    if _NC_CACHE is None:
        _NC_CACHE = _build_nc()
    return _NC_CACHE


def _prep_inputs(z1, z2):
    z1 = np.asarray(z1, dtype=np.float32)
    z2 = np.asarray(z2, dtype=np.float32)
    z2h = z2.astype(F16)
    z2t = np.ascontiguousarray(z2h.T)  # [128, N] fp16
    sq2 = (z2h.astype(np.float64) ** 2).sum(axis=-1)  # from the fp16 values
    base = (-A_SCALE * sq2)  # [N] float64
    # strip[r, c*512+u] = v[c*2048 + r*512 + u]
    def mkstrip(v):
        return np.ascontiguousarray(
            v.reshape(4, 4, 512).transpose(1, 0, 2).reshape(4, N // 4).astype(F16)
        )
    stripA = mkstrip(base.astype(np.float32))
    stripD = mkstrip((base + B_MAGIC).astype(np.float32))
    in_maps = []
    for c in range(NCORES):
        z1s = z1[c * SHARD : (c + 1) * SHARD]
        z1t2 = np.ascontiguousarray(
            (2.0 * A_SCALE * z1s.astype(np.float64)).astype(F16).T
        )
        in_maps.append(
            {"z1t2": z1t2, "z2t": z2t, "stripA": stripA, "stripD": stripD}
        )
    return in_maps


def _finish(z1, z2, res_list):
    dve = _dve_set()
    act_mask = np.zeros((ITILES, NGRAN), dtype=bool)
    for it in range(ITILES):
        for g in range(NGRAN):
            act_mask[it, g] = (it * 8 + g) not in dve
    rows_all = []
    for r in res_list:
        ra = np.asarray(r["rs_a"], np.float64).reshape(128, ITILES, NGRAN)
        rd = np.asarray(r["rs_d"], np.float64).reshape(128, ITILES)
        # zero out the garbage (DVE-lane) columns of rs_a, then add rs_d
        rows = (ra * act_mask[None, :, :]).sum(axis=2) + rd  # [128, ITILES]
        rows_all.append(rows.T.reshape(-1))  # row-major within shard
    rows = np.concatenate(rows_all)
    z1 = np.asarray(z1, dtype=np.float64)
    z2 = np.asarray(z2, dtype=np.float64)
    tdiag = 2.0 * (z1 * z2).sum(axis=-1) - (z2 * z2).sum(axis=-1)
    loss = np.mean(np.log(rows) - tdiag)
    return np.asarray(loss, dtype=np.float32)


def _ensure_hook_shim():
    """bass_utils imports antenv.axon_hooks whenever tracing is requested
    (e.g. via a BASS_TRACE env var); this image's antenv lacks that module.
    Provide an inert registry so tracing degrades to a warning instead of an
    ImportError.  A previously installed real shim is left untouched."""
    import sys

    try:
        import antenv.axon_hooks  # noqa: F401
    except ImportError:
        import types

        import antenv

        mod = types.ModuleType("antenv.axon_hooks")
        mod._hook = None
        mod.set_axon_ntff_profile_hook = lambda h: setattr(mod, "_hook", h)
        mod.get_axon_ntff_profile_hook = lambda: mod._hook
        sys.modules["antenv.axon_hooks"] = mod
        antenv.axon_hooks = mod


def _run(z1, z2, **spmd_kwargs):
    _ensure_hook_shim()
    from concourse.bass_utils import run_bass_kernel_spmd

    in_maps = _prep_inputs(z1, z2)
    res = run_bass_kernel_spmd(
        _get_nc(), in_maps, core_ids=list(range(NCORES)), **spmd_kwargs
    )
    return _finish(z1, z2, res.results), res


def kernel(z1, z2):
    loss, _ = _run(z1, z2)
    return loss
